# revision 11
# baseline (speedup 1.0000x reference)
"""BertScore model kernel for Trainium2 (8 NeuronCores, SPMD data-parallel over B).

Reference computation: cosine-normalized per-layer token reps, per-(layer,batch)
similarity matrix dots = h1 @ h2^T (256x256, contraction D=1024), ragged masked
max over rows/cols + masked means -> s1,s2, F1 harmonic mean -> (B,NL) features,
BatchNorm over batch, linear head -> (B,).

Design (v3.2):
- fp8 e4m3 inputs (h scaled by 32; |h|<=1 so max 32 << 240 TRN e4m3 limit)
  with DoubleRow matmuls: half the DMA bytes, half the PE matmul cycles vs fp16.
- Host planar layout: each SBUF partition reads ONE contiguous run per DMA.
- Ragged: the 64 batches are clustered into 8 SPMD slots (one batch per core
  per slot) sized to the cluster maxima (I_k, J_k) (~72% of the dense volume
  for the reference length distribution). The compiled program depends on the
  length arrays; builds are cached per slot-size tuple, so new length sets
  recompile but stay correct.
- No device-side masks: rows padded between the true length and the slot size
  are host-filled with DUPLICATES of token row 0, which can never change a
  row/column max; invalid entries are dropped in the host epilogue, which
  knows the true lengths.
- bf16 intermediate sim matrix; single fused ACT copy per (layer, slot);
  DVE max-reduces fused across layers AND 128-row chunks (2 per slot) to
  amortize per-instruction overheads; transposes deferred one step so the
  PE stream never waits inline on the ACT copy.
"""
import os
import numpy as np

NL, B, L1, L2, D = 4, 64, 256, 256, 1024
NCORES = 8
BB = B // NCORES          # batch slots per core
KT = D // 128             # contraction subtiles
SCALE = 32.0              # fp8 input scale; dots come back scaled by SCALE**2
BN_EPS = 1e-8
LOGIT_SCALE = 1.0

DTYPE = os.environ.get("BSM_DTYPE", "f8")        # f8 | f16
REPEAT = int(os.environ.get("BSM_REPEAT", "1"))  # body repeats (for timing)
DENSE = int(os.environ.get("BSM_DENSE", "0"))    # 1: pad all slots to 256
SKIP = set(os.environ.get("BSM_SKIP", "").split(","))  # debug: io,mm,act,red,dt
IOBUFS = int(os.environ.get("BSM_IOBUFS", "4"))
LOOPN = int(os.environ.get("BSM_LOOPN", "0"))  # >0: wrap body in device For_i loop

_CACHE = {}


def _build(dtype_name, repeat, iobufs, slots):
    """slots: tuple of (I_k, J_k) compile-time sizes for the BB batch slots."""
    import concourse.bacc as bacc
    import concourse.bass as bass
    import concourse.mybir as mybir
    import concourse.tile as tile
    from concourse.masks import make_identity

    f32 = mybir.dt.float32
    bf16 = mybir.dt.bfloat16
    dt_in = {"f8": mybir.dt.float8e4, "f16": mybir.dt.float16}[dtype_name]
    fp8 = dt_in == mybir.dt.float8e4

    nc = bacc.Bacc("TRN2", target_bir_lowering=False, debug=False,
                   num_devices=NCORES)

    # planar ragged pack, per partition p (contiguous, slot-major):
    #   [slot0: h1 (NL,KT,I_0) | h2 (NL,KT,J_0)][slot1: ...] ...
    # where element (t,l,q,i) of slot k is h_t[l, b_k, i, q*128+p] * SCALE
    offs = []
    W = 0
    for (I, J) in slots:
        offs.append(W)
        W += NL * KT * (I + J)
    hbd = nc.dram_tensor("hb", [128, W], dt_in, kind="ExternalInput")
    NCOL = NL * BB * 2
    rmd = nc.dram_tensor("rm", [128, NCOL], f32, kind="ExternalOutput")
    cmd = nc.dram_tensor("cm", [128, NCOL], f32, kind="ExternalOutput")

    with tile.TileContext(nc) as tc:
        with tc.tile_pool(name="consts", bufs=1) as consts, \
             tc.tile_pool(name="io", bufs=iobufs) as io, \
             tc.tile_pool(name="dsbp", bufs=3) as dsbp, \
             tc.tile_pool(name="accp", bufs=1) as accp, \
             tc.tile_pool(name="ps", bufs=3, space="PSUM") as ps, \
             tc.tile_pool(name="psT", bufs=2, space="PSUM") as psT:

            ident = consts.tile([128, 128], bf16)
            make_identity(nc, ident)

            RM = accp.tile([128, NCOL], f32)
            CM = accp.tile([128, NCOL], f32)
            if SKIP & {"io", "mm", "act", "red", "dt"}:
                nc.vector.memset(RM, 0.0)
                nc.vector.memset(CM, 0.0)

            hbap = hbd.ap()
            vmax = mybir.AluOpType.max
            X = mybir.AxisListType.X
            IDENT = mybir.ActivationFunctionType.Identity
            DR = mybir.MatmulPerfMode.DoubleRow

            import contextlib
            loop_cm = (tc.For_i(0, LOOPN, 1,
                                hint_engines=(mybir.EngineType.PE,))
                       if LOOPN > 0 else contextlib.nullcontext())

            def flush_transposes(pend):
                # deferred by one (l,slot) step so the PE stream never waits
                # inline on the ACT copy feeding the transposes
                if pend is None or "dt" in SKIP:
                    return
                dsb, k, l, ich, jch, I, J, last = pend
                dT = pend_dT[0]
                if dT is None:
                    dT = psT.tile([128, 2, NL, L1], bf16, tag="dT")
                    pend_dT[0] = dT
                for jt, cj in enumerate(jch):
                    for it, ci in enumerate(ich):
                        i0 = it * 128
                        nc.tensor.transpose(
                            out=dT[:cj, jt, l, i0:i0 + ci],
                            in_=dsb[:ci, it, l, jt * 128:jt * 128 + cj],
                            identity=ident[:ci, :ci])
                if last and "red" not in SKIP:
                    # col(l, jt) = (l*BB + k)*2 + jt
                    out = bass.AP(
                        tensor=CM.tensor, offset=CM.offset + k * 2,
                        ap=[CM.ap[0], [1, len(jch)], [2 * BB, NL]])
                    nc.vector.tensor_reduce(
                        out=out, in_=dT[:, :len(jch), :, :I],
                        axis=X, op=vmax)
                    pend_dT[0] = None

            with loop_cm:
              for _rep in range(repeat):
                pend = None
                pend_dT = [None]
                for k, (I, J) in enumerate(slots):
                    WK = NL * KT * (I + J)
                    hbt = io.tile([128, WK], dt_in, tag="hb")
                    if "io" not in SKIP:
                        o = offs[k]
                        nc.sync.dma_start(out=hbt, in_=hbap[:, o:o + WK])
                    h1v = hbt[:, :NL * KT * I].rearrange(
                        "p (l q i) -> p l q i", l=NL, q=KT)
                    h2v = hbt[:, NL * KT * I:].rearrange(
                        "p (l q j) -> p l q j", l=NL, q=KT)
                    ich = [min(128, I)] + ([I - 128] if I > 128 else [])
                    jch = [min(128, J)] + ([J - 128] if J > 128 else [])
                    dsb = dsbp.tile([128, 2, NL, L2], bf16, tag="dsb")
                    for l in range(NL):
                        if "mm" in SKIP:
                            continue
                        dps = ps.tile([128, 2, L2], f32, tag="dots")
                        for it, ci in enumerate(ich):
                            i0 = it * 128
                            if fp8:
                                for qp in range(0, KT, 2):
                                    nc.tensor.matmul(
                                        out=dps[:ci, it, :J],
                                        lhsT=h1v[:, l, qp:qp + 2, i0:i0 + ci],
                                        rhs=h2v[:, l, qp:qp + 2, :],
                                        start=(qp == 0), stop=(qp == KT - 2),
                                        perf_mode=DR)
                            else:
                                for q in range(KT):
                                    nc.tensor.matmul(
                                        out=dps[:ci, it, :J],
                                        lhsT=h1v[:, l, q, i0:i0 + ci],
                                        rhs=h2v[:, l, q, :],
                                        start=(q == 0), stop=(q == KT - 1))
                        if "act" in SKIP:
                            continue
                        # single fused PSUM->SBUF bf16 copy (both 128-chunks)
                        nc.scalar.activation(
                            out=dsb[:, :len(ich), l, :J],
                            in_=dps[:, :len(ich), :J], func=IDENT)
                        if l == NL - 1 and "red" not in SKIP:
                            # row max over j, fused over chunks and layers:
                            # col(l, it) = (l*BB + k)*2 + it
                            import concourse.bass as bass_
                            out = bass_.AP(
                                tensor=RM.tensor, offset=RM.offset + k * 2,
                                ap=[RM.ap[0], [1, len(ich)], [2 * BB, NL]])
                            nc.vector.tensor_reduce(
                                out=out, in_=dsb[:, :len(ich), :, :J],
                                axis=X, op=vmax)
                        flush_transposes(pend)
                        pend = (dsb, k, l, ich, jch, I, J, l == NL - 1)
                flush_transposes(pend)

            nc.sync.dma_start(out=rmd.ap(), in_=RM)
            nc.sync.dma_start(out=cmd.ap(), in_=CM)

    nc.finalize()
    return nc


def _assign_slots(len1, len2):
    """Cluster the B batches into BB slots of NCORES members, minimizing
    sum over slots of (max len1 + max len2). Returns (perm, slots):
    perm[k][c] = original batch index at (core c, slot k)."""
    import itertools
    l1 = np.asarray(len1).astype(int)
    l2 = np.asarray(len2).astype(int)
    if DENSE:
        perm = [[k * NCORES + c for c in range(NCORES)] for k in range(BB)]
        return perm, [(L1, L2)] * BB
    order = np.argsort(-(l1 + l2))
    groups = [list(order[NCORES * g:NCORES * (g + 1)]) for g in range(BB)]

    def gcost(g):
        return l1[g].max() + l2[g].max()

    improved = True
    while improved:
        improved = False
        for ga, gb in itertools.combinations(range(BB), 2):
            ca, cb = gcost(groups[ga]), gcost(groups[gb])
            for i in range(NCORES):
                for j in range(NCORES):
                    groups[ga][i], groups[gb][j] = groups[gb][j], groups[ga][i]
                    c = gcost(groups[ga]) + gcost(groups[gb])
                    if c < ca + cb - 1e-9:
                        ca, cb = gcost(groups[ga]), gcost(groups[gb])
                        improved = True
                    else:
                        groups[ga][i], groups[gb][j] = \
                            groups[gb][j], groups[ga][i]

    def rnd(x, m):
        # DoubleRow ldweights reject partial widths off the 32-boundary;
        # moving-side widths are kept to multiples of 8
        return min(256, (int(x) + m - 1) & ~(m - 1))

    slots = [(rnd(l1[g].max(), 32), rnd(l2[g].max(), 8)) for g in groups]
    return [list(map(int, g)) for g in groups], slots


def _get_nc(slots):
    key = (DTYPE, REPEAT, IOBUFS, LOOPN, tuple(sorted(SKIP)), tuple(slots))
    if key not in _CACHE:
        _CACHE[key] = _build(DTYPE, REPEAT, IOBUFS, tuple(slots))
    return _CACHE[key]


def _host_prep(reps1, reps2, len1, len2, perm, slots):
    """Normalize+scale, pack the ragged planar fp8 array per core.
    Rows in [len, slotmax) duplicate token row 0 (max-neutral padding)."""
    import ml_dtypes
    np_in = {"f8": ml_dtypes.float8_e4m3, "f16": np.float16}[DTYPE]

    def planar(r):
        r = np.asarray(r, dtype=np.float32)
        n = np.sqrt(np.einsum('lbid,lbid->lbi', r, r))
        h = r * (SCALE / n[..., None])                # (NL, B, L, D)
        x = h.reshape(NL, B, L1, KT, 128)             # d = q*128 + p
        return x.transpose(4, 1, 0, 3, 2).astype(np_in)   # (128, B, NL, KT, L)

    p1 = planar(reps1)
    p2 = planar(reps2)
    len1 = np.asarray(len1).astype(np.int64)
    len2 = np.asarray(len2).astype(np.int64)

    W = sum(NL * KT * (I + J) for (I, J) in slots)
    in_maps = []
    for c in range(NCORES):
        hb = np.empty((128, W), dtype=np_in)
        o = 0
        for k, (I, J) in enumerate(slots):
            b = perm[k][c]
            for p, n, ln in ((p1, I, len1[b]), (p2, J, len2[b])):
                s = p[:, b, :, :, :n].copy()          # (128, NL, KT, n)
                s[:, :, :, ln:] = s[:, :, :, :1]      # duplicate row 0
                hb[:, o:o + NL * KT * n] = s.reshape(128, NL * KT * n)
                o += NL * KT * n
        in_maps.append({"hb": hb})
    return in_maps, len1, len2


def _epilogue(results, len1, len2, w, b, perm):
    """rm/cm (128, NL*BB*2) per core -> s1,s2 -> F1 -> BatchNorm -> head."""
    maxv_rows = np.empty((NL, B, L1), dtype=np.float64)  # max over valid j, per i
    maxv_cols = np.empty((NL, B, L2), dtype=np.float64)  # max over valid i, per j
    for c, res in enumerate(results):
        rm = np.asarray(res["rm"], dtype=np.float64)  # (128, NCOL)
        cm = np.asarray(res["cm"], dtype=np.float64)
        # column t = (l*BB + k)*2 + half ; partition p -> index half*128 + p
        rm_r = rm.T.reshape(NL, BB, 2, 128).reshape(NL, BB, 256)
        cm_r = cm.T.reshape(NL, BB, 2, 128).reshape(NL, BB, 256)
        for k in range(BB):
            bidx = perm[k][c]
            maxv_rows[:, bidx] = rm_r[:, k]
            maxv_cols[:, bidx] = cm_r[:, k]
    inv = 1.0 / (SCALE * SCALE)
    maxv_rows *= inv
    maxv_cols *= inv

    ar1 = np.arange(L1)[None, :]
    ar2 = np.arange(L2)[None, :]
    mask1 = (ar1 < len1[:, None])  # (B, L1)
    mask2 = (ar2 < len2[:, None])
    n1 = len1.astype(np.float64)
    n2 = len2.astype(np.float64)

    # s2: mean over valid i of (max over valid j); s1: mean over valid j of
    # (max over valid i)
    with np.errstate(invalid="ignore"):
        s2 = np.where(mask1[None], maxv_rows, 0.0).sum(axis=2) / n1[None]
        s1 = np.where(mask2[None], maxv_cols, 0.0).sum(axis=2) / n2[None]
    feat = (2.0 * s1 * s2 / (s1 + s2)).T                    # (B, NL)
    mean = feat.mean(axis=0, keepdims=True)
    var = ((feat - mean) ** 2).mean(axis=0, keepdims=True)
    feat = (feat - mean) / np.sqrt(var + BN_EPS)
    w = np.asarray(w, dtype=np.float64)
    bb = np.asarray(b, dtype=np.float64)
    out = LOGIT_SCALE * (feat @ w.T + bb)[:, 0]
    return out.astype(np.float32)


LAST_RUN = {}


def kernel(reps1, reps2, len1, len2, w, b):
    from concourse.bass_utils import run_bass_kernel_spmd

    perm, slots = _assign_slots(len1, len2)
    nc = _get_nc(slots)
    in_maps, l1, l2 = _host_prep(reps1, reps2, len1, len2, perm, slots)
    res = run_bass_kernel_spmd(nc, in_maps, list(range(NCORES)))
    LAST_RUN["results"] = res
    LAST_RUN["in_maps"] = in_maps
    LAST_RUN["nc"] = nc
    LAST_RUN["slots"] = slots
    return _epilogue(res.results, l1, l2, w, b, perm)


# revision 14
# speedup vs baseline: 2.1200x; 2.1200x over previous
"""BertScore model kernel for Trainium2 (8 NeuronCores, SPMD data-parallel over B).

Reference computation: cosine-normalized per-layer token reps, per-(layer,batch)
similarity matrix dots = h1 @ h2^T (256x256, contraction D=1024), ragged masked
max over rows/cols + masked means -> s1,s2, F1 harmonic mean -> (B,NL) features,
BatchNorm over batch, linear head -> (B,).

Design (v3.2):
- fp8 e4m3 inputs (h scaled by 32; |h|<=1 so max 32 << 240 TRN e4m3 limit)
  with DoubleRow matmuls: half the DMA bytes, half the PE matmul cycles vs fp16.
- Host planar layout: each SBUF partition reads ONE contiguous run per DMA.
- Ragged: the 64 batches are clustered into 8 SPMD slots (one batch per core
  per slot) sized to the cluster maxima (I_k, J_k) (~72% of the dense volume
  for the reference length distribution). The compiled program depends on the
  length arrays; builds are cached per slot-size tuple, so new length sets
  recompile but stay correct.
- No device-side masks: rows padded between the true length and the slot size
  are host-filled with DUPLICATES of token row 0, which can never change a
  row/column max; invalid entries are dropped in the host epilogue, which
  knows the true lengths.
- bf16 intermediate sim matrix; single fused ACT copy per (layer, slot);
  DVE max-reduces fused across layers AND 128-row chunks (2 per slot) to
  amortize per-instruction overheads; transposes deferred one step so the
  PE stream never waits inline on the ACT copy.
"""
import os
import numpy as np

NL, B, L1, L2, D = 4, 64, 256, 256, 1024
NCORES = 8
BB = B // NCORES          # batch slots per core
KT = D // 128             # contraction subtiles
SCALE = 32.0              # fp8 input scale; dots come back scaled by SCALE**2
BN_EPS = 1e-8
LOGIT_SCALE = 1.0

DTYPE = os.environ.get("BSM_DTYPE", "f8")        # f8 | f16
REPEAT = int(os.environ.get("BSM_REPEAT", "1"))  # body repeats (for timing)
DENSE = int(os.environ.get("BSM_DENSE", "0"))    # 1: pad all slots to 256
SKIP = set(os.environ.get("BSM_SKIP", "").split(","))  # debug: io,mm,act,red,dt
IOBUFS = int(os.environ.get("BSM_IOBUFS", "3"))
LOOPN = int(os.environ.get("BSM_LOOPN", "0"))  # >0: wrap body in device For_i loop

_CACHE = {}


def _build(dtype_name, repeat, iobufs, slots):
    """slots: tuple of (I_k, J_k) compile-time sizes for the BB batch slots."""
    import concourse.bacc as bacc
    import concourse.bass as bass
    import concourse.mybir as mybir
    import concourse.tile as tile
    from concourse.masks import make_identity

    f32 = mybir.dt.float32
    bf16 = mybir.dt.bfloat16
    dt_in = {"f8": mybir.dt.float8e4, "f16": mybir.dt.float16}[dtype_name]
    fp8 = dt_in == mybir.dt.float8e4

    nc = bacc.Bacc("TRN2", target_bir_lowering=False, debug=False,
                   num_devices=NCORES)

    # planar ragged pack, per partition p (contiguous, slot-major):
    #   [slot0: h1 (NL,KT,I_0) | h2 (NL,KT,J_0)][slot1: ...] ...
    # where element (t,l,q,i) of slot k is h_t[l, b_k, i, q*128+p] * SCALE
    offs = []
    W = 0
    for (I, J) in slots:
        offs.append(W)
        W += NL * KT * (I + J)
    hbd = nc.dram_tensor("hb", [128, W], dt_in, kind="ExternalInput")
    NCOL = NL * BB * 2
    rmd = nc.dram_tensor("rm", [128, NCOL], f32, kind="ExternalOutput")
    cmd = nc.dram_tensor("cm", [128, NCOL], f32, kind="ExternalOutput")

    with tile.TileContext(nc) as tc:
        with tc.tile_pool(name="consts", bufs=1) as consts, \
             tc.tile_pool(name="io", bufs=iobufs) as io, \
             tc.tile_pool(name="dsbp", bufs=4) as dsbp, \
             tc.tile_pool(name="accp", bufs=1) as accp, \
             tc.tile_pool(name="ps", bufs=3, space="PSUM") as ps, \
             tc.tile_pool(name="psT", bufs=2, space="PSUM") as psT:

            ident = consts.tile([128, 128], bf16)
            make_identity(nc, ident)

            RM = accp.tile([128, NCOL], f32)
            CM = accp.tile([128, NCOL], f32)
            if SKIP & {"io", "mm", "act", "red", "dt"}:
                nc.vector.memset(RM, 0.0)
                nc.vector.memset(CM, 0.0)

            hbap = hbd.ap()
            vmax = mybir.AluOpType.max
            X = mybir.AxisListType.X
            IDENT = mybir.ActivationFunctionType.Identity
            DR = mybir.MatmulPerfMode.DoubleRow

            import contextlib
            loop_cm = (tc.For_i(0, LOOPN, 1,
                                hint_engines=(mybir.EngineType.PE,))
                       if LOOPN > 0 else contextlib.nullcontext())

            with loop_cm:
              for _rep in range(repeat):
                for k, (I, J) in enumerate(slots):
                    WK = NL * KT * (I + J)
                    hbt = io.tile([128, WK], dt_in, tag="hb")
                    if "io" not in SKIP:
                        o = offs[k]
                        nc.sync.dma_start(out=hbt, in_=hbap[:, o:o + WK])
                    h1v = hbt[:, :NL * KT * I].rearrange(
                        "p (l q i) -> p l q i", l=NL, q=KT)
                    h2v = hbt[:, NL * KT * I:].rearrange(
                        "p (l q j) -> p l q j", l=NL, q=KT)
                    ich = [min(128, I)] + ([I - 128] if I > 128 else [])
                    jch = [min(128, J)] + ([J - 128] if J > 128 else [])
                    for l in range(NL):
                        if "mm" in SKIP:
                            continue
                        dps = ps.tile([128, 2, L2], f32, tag="dots")
                        for it, ci in enumerate(ich):
                            i0 = it * 128
                            if fp8:
                                for qp in range(0, KT, 2):
                                    nc.tensor.matmul(
                                        out=dps[:ci, it, :J],
                                        lhsT=h1v[:, l, qp:qp + 2, i0:i0 + ci],
                                        rhs=h2v[:, l, qp:qp + 2, :],
                                        start=(qp == 0), stop=(qp == KT - 2),
                                        perf_mode=DR)
                            else:
                                for q in range(KT):
                                    nc.tensor.matmul(
                                        out=dps[:ci, it, :J],
                                        lhsT=h1v[:, l, q, i0:i0 + ci],
                                        rhs=h2v[:, l, q, :],
                                        start=(q == 0), stop=(q == KT - 1))
                        if "act" in SKIP:
                            continue
                        # single fused PSUM->SBUF bf16 copy (both 128-chunks)
                        dsb = dsbp.tile([128, 2, L2], bf16, tag="dsb")
                        nc.scalar.activation(
                            out=dsb[:, :len(ich), :J],
                            in_=dps[:, :len(ich), :J], func=IDENT)
                        col = (l * BB + k) * 2
                        if "red" not in SKIP:
                            # row max over j, both 128-halves of i at once
                            nc.vector.tensor_reduce(
                                out=RM[:, col:col + len(ich)],
                                in_=dsb[:, :len(ich), :J], axis=X, op=vmax)
                        if "dt" in SKIP:
                            continue
                        dT = psT.tile([128, 2, L1], bf16, tag="dT")
                        for jt, cj in enumerate(jch):
                            for it, ci in enumerate(ich):
                                i0 = it * 128
                                nc.tensor.transpose(
                                    out=dT[:cj, jt, i0:i0 + ci],
                                    in_=dsb[:ci, it, jt * 128:jt * 128 + cj],
                                    identity=ident[:ci, :ci])
                        if "red" not in SKIP:
                            nc.vector.tensor_reduce(
                                out=CM[:, col:col + len(jch)],
                                in_=dT[:, :len(jch), :I], axis=X, op=vmax)

            nc.sync.dma_start(out=rmd.ap(), in_=RM)
            nc.sync.dma_start(out=cmd.ap(), in_=CM)

    nc.finalize()
    return nc


def _assign_slots(len1, len2):
    """Cluster the B batches into BB slots of NCORES members, minimizing
    sum over slots of (max len1 + max len2). Returns (perm, slots):
    perm[k][c] = original batch index at (core c, slot k)."""
    import itertools
    l1 = np.asarray(len1).astype(int)
    l2 = np.asarray(len2).astype(int)
    if DENSE:
        perm = [[k * NCORES + c for c in range(NCORES)] for k in range(BB)]
        return perm, [(L1, L2)] * BB
    order = np.argsort(-(l1 + l2))
    groups = [list(order[NCORES * g:NCORES * (g + 1)]) for g in range(BB)]

    def gcost(g):
        return l1[g].max() + l2[g].max()

    improved = True
    while improved:
        improved = False
        for ga, gb in itertools.combinations(range(BB), 2):
            ca, cb = gcost(groups[ga]), gcost(groups[gb])
            for i in range(NCORES):
                for j in range(NCORES):
                    groups[ga][i], groups[gb][j] = groups[gb][j], groups[ga][i]
                    c = gcost(groups[ga]) + gcost(groups[gb])
                    if c < ca + cb - 1e-9:
                        ca, cb = gcost(groups[ga]), gcost(groups[gb])
                        improved = True
                    else:
                        groups[ga][i], groups[gb][j] = \
                            groups[gb][j], groups[ga][i]

    def rnd(x, m):
        # DoubleRow ldweights reject partial widths off the 32-boundary;
        # moving-side widths are kept to multiples of 8
        return min(256, (int(x) + m - 1) & ~(m - 1))

    slots = [(rnd(l1[g].max(), 32), rnd(l2[g].max(), 8)) for g in groups]
    return [list(map(int, g)) for g in groups], slots


def _get_nc(slots):
    key = (DTYPE, REPEAT, IOBUFS, LOOPN, tuple(sorted(SKIP)), tuple(slots))
    if key not in _CACHE:
        _CACHE[key] = _build(DTYPE, REPEAT, IOBUFS, tuple(slots))
    return _CACHE[key]


def _host_prep(reps1, reps2, len1, len2, perm, slots):
    """Normalize+scale, pack the ragged planar fp8 array per core.
    Rows in [len, slotmax) duplicate token row 0 (max-neutral padding)."""
    import ml_dtypes
    np_in = {"f8": ml_dtypes.float8_e4m3, "f16": np.float16}[DTYPE]

    def planar(r):
        r = np.asarray(r, dtype=np.float32)
        n = np.sqrt(np.einsum('lbid,lbid->lbi', r, r))
        h = r * (SCALE / n[..., None])                # (NL, B, L, D)
        x = h.reshape(NL, B, L1, KT, 128)             # d = q*128 + p
        return x.transpose(4, 1, 0, 3, 2).astype(np_in)   # (128, B, NL, KT, L)

    p1 = planar(reps1)
    p2 = planar(reps2)
    len1 = np.asarray(len1).astype(np.int64)
    len2 = np.asarray(len2).astype(np.int64)

    W = sum(NL * KT * (I + J) for (I, J) in slots)
    in_maps = []
    for c in range(NCORES):
        hb = np.empty((128, W), dtype=np_in)
        o = 0
        for k, (I, J) in enumerate(slots):
            b = perm[k][c]
            for p, n, ln in ((p1, I, len1[b]), (p2, J, len2[b])):
                s = p[:, b, :, :, :n].copy()          # (128, NL, KT, n)
                s[:, :, :, ln:] = s[:, :, :, :1]      # duplicate row 0
                hb[:, o:o + NL * KT * n] = s.reshape(128, NL * KT * n)
                o += NL * KT * n
        in_maps.append({"hb": hb})
    return in_maps, len1, len2


def _epilogue(results, len1, len2, w, b, perm):
    """rm/cm (128, NL*BB*2) per core -> s1,s2 -> F1 -> BatchNorm -> head."""
    maxv_rows = np.empty((NL, B, L1), dtype=np.float64)  # max over valid j, per i
    maxv_cols = np.empty((NL, B, L2), dtype=np.float64)  # max over valid i, per j
    for c, res in enumerate(results):
        rm = np.asarray(res["rm"], dtype=np.float64)  # (128, NCOL)
        cm = np.asarray(res["cm"], dtype=np.float64)
        # column t = (l*BB + k)*2 + half ; partition p -> index half*128 + p
        rm_r = rm.T.reshape(NL, BB, 2, 128).reshape(NL, BB, 256)
        cm_r = cm.T.reshape(NL, BB, 2, 128).reshape(NL, BB, 256)
        for k in range(BB):
            bidx = perm[k][c]
            maxv_rows[:, bidx] = rm_r[:, k]
            maxv_cols[:, bidx] = cm_r[:, k]
    inv = 1.0 / (SCALE * SCALE)
    maxv_rows *= inv
    maxv_cols *= inv

    ar1 = np.arange(L1)[None, :]
    ar2 = np.arange(L2)[None, :]
    mask1 = (ar1 < len1[:, None])  # (B, L1)
    mask2 = (ar2 < len2[:, None])
    n1 = len1.astype(np.float64)
    n2 = len2.astype(np.float64)

    # s2: mean over valid i of (max over valid j); s1: mean over valid j of
    # (max over valid i)
    with np.errstate(invalid="ignore"):
        s2 = np.where(mask1[None], maxv_rows, 0.0).sum(axis=2) / n1[None]
        s1 = np.where(mask2[None], maxv_cols, 0.0).sum(axis=2) / n2[None]
    feat = (2.0 * s1 * s2 / (s1 + s2)).T                    # (B, NL)
    mean = feat.mean(axis=0, keepdims=True)
    var = ((feat - mean) ** 2).mean(axis=0, keepdims=True)
    feat = (feat - mean) / np.sqrt(var + BN_EPS)
    w = np.asarray(w, dtype=np.float64)
    bb = np.asarray(b, dtype=np.float64)
    out = LOGIT_SCALE * (feat @ w.T + bb)[:, 0]
    return out.astype(np.float32)


LAST_RUN = {}


def kernel(reps1, reps2, len1, len2, w, b):
    from concourse.bass_utils import run_bass_kernel_spmd

    perm, slots = _assign_slots(len1, len2)
    nc = _get_nc(slots)
    in_maps, l1, l2 = _host_prep(reps1, reps2, len1, len2, perm, slots)
    res = run_bass_kernel_spmd(nc, in_maps, list(range(NCORES)))
    LAST_RUN["results"] = res
    LAST_RUN["in_maps"] = in_maps
    LAST_RUN["nc"] = nc
    LAST_RUN["slots"] = slots
    return _epilogue(res.results, l1, l2, w, b, perm)
